# revision 75
# baseline (speedup 1.0000x reference)
"""BinaryAttention on 8 TRN2 NeuronCores (Bass/Tile, SPMD tensor-parallel).

Math (per reference):
  Wb = alpha * sign(W), alpha[o] = mean_c |W[o,c]|
  q/k/v = x @ Wb_{q,k,v}^T + b;   att = softmax(q k^T / sqrt(Dh));
  y = att @ v;  out = y @ Wb_p^T + bp

Sharding (8 cores):
  - Heads (16) sharded 2/core: each core computes q,k,v for its 2 heads over
    all (B,T) and runs attention for them, producing its 128 y-channels.
  - Proj is token-sharded: core i owns a 256-token half-tile per batch. One
    AllToAll per batch routes each y element to exactly its owner (2MB/core
    total instead of a 16MB/core AllGather replication), then each core
    projects its own tokens against the full binarized Wp.

Score matmuls run with K=128 by keeping k in a zero-padded per-head layout
(k_pad[:, h] has k_h in rows h*64..h*64+63, zeros elsewhere): the PE streams
~2x faster at K>=96 than at K=64, and the zero rows annihilate the other
head's q rows. q/k/scores stay bf16; x, the qkv sign weights, exp(att) and
v are fp8e4 so the QKV and AV matmuls run K=256 fp8 DoubleRow (~2x PE
throughput; rel err ~1.6e-2 vs the 2e-2 budget). alpha/bias applied in fp32
on PSUM. Softmax skips max-subtraction (scores are O(1) here). Binarization
(sign/alpha) is precomputed on the host.
"""

import numpy as np
import ml_dtypes

import concourse.bass as bass
import concourse.bacc as bacc
import concourse.tile as tile
from concourse import mybir
from concourse.masks import make_identity
from concourse.bass_utils import run_bass_kernel_spmd

NC = 8          # cores
B, T, C = 4, 2048, 1024
H, DH = 16, 64
HPC = H // NC   # heads per core = 2
OS = HPC * DH   # per-core o-slice width = 128
KC = C // 128   # contraction chunks = 8
NTOK = B * T    # 8192
NT = 512        # moving-operand tile (fp32 psum bank)
HT = NT // 2    # half-tile tokens owned per core per batch = 256
SCALE = DH ** -0.5

F32 = mybir.dt.float32
BF16 = mybir.dt.bfloat16
FP8 = mybir.dt.float8e4
DR = mybir.MatmulPerfMode.DoubleRow

_CACHED = {}


def _build():
    nc = bacc.Bacc("TRN2", target_bir_lowering=False, debug=False, num_devices=NC)

    xH = nc.dram_tensor("xH", [NTOK // NT, 128, KC, NT], FP8,
                        kind="ExternalInput")
    sq_d = nc.dram_tensor("sq", [128, KC, OS], FP8, kind="ExternalInput")
    sk_d = nc.dram_tensor("sk", [128, KC, OS], FP8, kind="ExternalInput")
    sv_d = nc.dram_tensor("sv", [128, KC, OS], FP8, kind="ExternalInput")
    # full binarized proj weight: [128(c%128), c-block, o-block, 128(o%128)]
    sp_d = nc.dram_tensor("sp", [128, KC, KC, 128], BF16, kind="ExternalInput")
    ab_d = nc.dram_tensor("ab", [128, 6], F32, kind="ExternalInput")
    ap_d = nc.dram_tensor("apf", [128, KC], F32, kind="ExternalInput")
    bp_d = nc.dram_tensor("bpf", [128, KC], F32, kind="ExternalInput")
    # token-sharded output: per batch, this core's 256 tokens x all 1024 outs.
    # Batches 0..2: one 256-token half-tile (tt=i//2, half=i%2). Batch 3 is
    # exchanged as two quarter-tile AllToAlls (shorter tail): this core owns
    # 128-token quarter i%4 of tile i//4 in each half of the batch.
    out_t = nc.dram_tensor("out_t", [B, C, HT], F32, kind="ExternalOutput")

    with tile.TileContext(nc, num_cores=NC) as tc:
        with (
            tc.tile_pool(name="const", bufs=1) as const,
            tc.tile_pool(name="xin", bufs=5) as xin,
            tc.tile_pool(name="qkv", bufs=2) as qkvp,
            tc.tile_pool(name="attp", bufs=9) as attp,
            tc.tile_pool(name="ypool", bufs=8) as ypool,
            tc.tile_pool(name="ysb", bufs=4) as ysbp,
            tc.tile_pool(name="ygp", bufs=3) as ygp,
            tc.tile_pool(name="outp", bufs=2) as outp,
            tc.tile_pool(name="mm_ps", bufs=2, space="PSUM") as mm_ps,
            tc.tile_pool(name="sc_ps", bufs=2, space="PSUM") as sc_ps,
            tc.tile_pool(name="y_ps", bufs=2, space="PSUM") as y_ps,
            tc.tile_pool(name="dram", bufs=1, space="DRAM") as dram,
        ):
            # ---------------- prologue: load host-binarized weights --------
            # startup is DMA-latency-bound: interleave sq and x(0,0) in
            # single-kc chunks round-robin over 4 queues so QKV matmul kc=0
            # can fire as soon as ~160KB lands instead of ~770KB
            signs = {}
            for wn, s_d in (("q", sq_d), ("k", sk_d), ("v", sv_d)):
                s_sb = const.tile([128, KC, OS], FP8, name=f"s_{wn}", tag=f"s_{wn}")
                signs[wn] = s_sb
            ab_sb = const.tile([128, 6], F32, tag="ab")
            nc.scalar.dma_start(ab_sb[:], ab_d[:])
            _abcol = {"q": 0, "k": 1, "v": 2}

            def alpha_ap(w, p0=0, p1=128):
                c = _abcol[w]
                return ab_sb[p0:p1, c:c + 1]

            def bias_ap(w, p0=0, p1=128):
                c = 3 + _abcol[w]
                return ab_sb[p0:p1, c:c + 1]

            ident = const.tile([128, 128], BF16, tag="ident")
            make_identity(nc, ident)

            x_cache = {}
            _q4 = [nc.sync, nc.gpsimd, nc.scalar]

            def _load_x(b, nt):
                x_sb = xin.tile([128, KC, NT], FP8, name=f"x_{b}_{nt}", tag="x")
                ti = b * (T // NT) + nt
                if (b, nt) == (0, 0):
                    # emitted inline with the sq chunks below
                    pass
                elif b == 0:
                    # half-tile per queue: first x usable in ~7us not ~13
                    nc.sync.dma_start(
                        x_sb[:, 0:KC // 2, :], xH[ti, :, 0:KC // 2, :])
                    nc.gpsimd.dma_start(
                        x_sb[:, KC // 2:, :], xH[ti, :, KC // 2:, :])
                else:
                    q = nc.sync if nt % 2 == 0 else nc.scalar
                    q.dma_start(x_sb[:], xH[ti])
                x_cache[(b, nt)] = x_sb
                return x_sb

            x00 = _load_x(0, 0)
            for kc in range(KC):
                eng = _q4[kc % 3]
                eng.dma_start(
                    signs["q"][:, kc:kc + 1, :], sq_d[:, kc:kc + 1, :])
                eng.dma_start(x00[:, kc:kc + 1, :], xH[0, :, kc:kc + 1, :])
            # k signs next (needed ~2nd in the QKV pipeline), split 3-way
            for i, (c0, c1) in enumerate(((0, 3), (3, 6), (6, 8))):
                _q4[i].dma_start(
                    signs["k"][:, c0:c1, :], sk_d[:, c0:c1, :])
            # remaining batch-0 tiles in kc-thirds across all three queues
            # so k(0,1..3) aren't starved behind two-queue half-tile loads
            for nt in range(1, T // NT):
                x_sb = xin.tile([128, KC, NT], FP8, name=f"x_0_{nt}", tag="x")
                for i, (c0, c1) in enumerate(((0, 3), (3, 6), (6, 8))):
                    _q4[(nt + i) % 3].dma_start(
                        x_sb[:, c0:c1, :], xH[nt, :, c0:c1, :])
                x_cache[(0, nt)] = x_sb
            nc.scalar.dma_start(
                signs["v"][:, 0:KC // 2, :], sv_d[:, 0:KC // 2, :])
            nc.scalar.dma_start(
                signs["v"][:, KC // 2:, :], sv_d[:, KC // 2:, :])

            # tiny warmup collective: the first CC op pays ~25us of ring
            # setup; spend it here while the PE chews on batch-0 QKV
            wu_in = dram.tile([NC, 8], BF16, name="wu_i", tag="wu_i")
            wu_out = dram.tile([NC, 8], BF16, name="wu_o", tag="wu_o")
            wu_sb = const.tile([1, NC * 8], BF16, tag="wu")
            nc.vector.memset(wu_sb[:], 0.0)
            nc.gpsimd.dma_start(wu_in.rearrange("r t -> (r t)")[None, :], wu_sb[:])
            nc.gpsimd.collective_compute(
                "AllToAll", mybir.AluOpType.bypass,
                replica_groups=[list(range(NC))],
                ins=[wu_in.opt()], outs=[wu_out.opt()],
            )

            # proj weights arrive behind the warmup on the gpsimd queue
            # (not needed until ~150us); keeps the sync queue free for x
            sp_sb = const.tile([128, KC, KC, 128], BF16, tag="s_p")
            nc.scalar.dma_start(sp_sb[:], sp_d[:])
            ap_sb = const.tile([128, KC], F32, tag="a_p")
            nc.scalar.dma_start(ap_sb[:], ap_d[:])
            bp_sb = const.tile([128, KC], F32, tag="b_p")
            nc.scalar.dma_start(bp_sb[:], bp_d[:])

            # ------------- token-sharded proj via AllToAll -----------------
            # groups: (b, 0) for b<3 are full batches (half-tile blocks of
            # 256 tokens); batch 3 exchanges per-tile (64-token slots) so
            # the final, unhidable exchange is as small as possible.
            a2a_bufs = {}
            for b in range(B - 1):
                y_d = dram.tile([NC, 128, HT], BF16, name=f"yd{b}", tag=f"yd{b}")
                yg_d = dram.tile([NC, 128, HT], BF16, name=f"ygd{b}", tag=f"ygd{b}")
                a2a_bufs[(b, 0)] = (y_d, yg_d, HT)
            # batch 3: tiles 0+1 in one group, then tiles 2 and 3 alone so
            # the CC pipe is idle when the final (smallest) exchange fires
            for g, w3 in ((0, HT // 2), (1, NT // NC), (2, NT // NC)):
                y_d = dram.tile([NC, 128, w3], BF16, name=f"yd3{g}",
                                tag=f"yd3{g}")
                yg_d = dram.tile([NC, 128, w3], BF16, name=f"ygd3{g}",
                                 tag=f"ygd3{g}")
                a2a_bufs[(3, g)] = (y_d, yg_d, w3)
            y_d_written = {b: 0 for b in range(B)}

            def proj_a2a(b, g, load=True):
                y_d, yg_d, w = a2a_bufs[(b, g)]
                nc.gpsimd.collective_compute(
                    "AllToAll", mybir.AluOpType.bypass,
                    replica_groups=[list(range(NC))],
                    ins=[y_d.opt()], outs=[yg_d.opt()],
                )
                if not load:
                    return None
                yg_sb = ygp.tile([128, NC, w], BF16, name=f"yg{b}{g}", tag="yg")
                # yg loads wait on their collective — keep them off gpsimd
                # where they would head-of-line-block later CC triggers and
                # the tail norm chain
                nc.sync.dma_start(
                    yg_sb[:], yg_d.rearrange("r p t -> p r t"))
                return yg_sb

            def proj_mm(b, g, yg_sb):
                _, _, w = a2a_bufs[(b, g)]
                t0 = 0 if b < 3 else (0, HT // 2, HT // 2 + NT // NC)[g]
                orr = out_t[b, :, t0:t0 + w].rearrange("(m p) t -> p m t", p=128)
                o_sb = outp.tile([128, KC, w], F32, name=f"ob{b}{g}", tag="ob")
                for m in range(KC):
                    pp = mm_ps.tile([128, w], F32, name=f"pp{b}{g}{m}", tag="mm")
                    for r in range(NC):
                        nc.tensor.matmul(
                            pp[:], sp_sb[:, r, m, :], yg_sb[:, r, :],
                            start=(r == 0), stop=(r == NC - 1),
                        )
                    nc.vector.tensor_scalar(
                        out=o_sb[:, m, :], in0=pp[:],
                        scalar1=ap_sb[:, m:m + 1], scalar2=bp_sb[:, m:m + 1],
                        op0=mybir.AluOpType.mult, op1=mybir.AluOpType.add,
                    )
                if (b, g) == (3, 2):
                    nc.gpsimd.dma_start(orr[:, 0:KC // 2], o_sb[:, 0:KC // 2])
                    nc.sync.dma_start(orr[:, KC // 2:], o_sb[:, KC // 2:])
                else:
                    # out DMAs ride sync: scalar must be free at the tail for
                    # the last tile's norm chain
                    nc.sync.dma_start(orr, o_sb[:])

            a2a_done = {}

            pend_norm = []

            # all-ones stationary for the tail-norm PE broadcast (only the
            # partition-64 row is read)
            ones_bc = const.tile([DH + 1, DH], F32, tag="ones_bc")
            nc.vector.memset(ones_bc[:], 1.0)

            def emit_norm(item):
                b, tt, h, t0, yc, y_sb = item
                if (b, tt) == (B - 1, T // NT - 1):
                    # last tile: the whole norm chain is on the critical tail
                    # before the final AllToAll. SBUF->SBUF fold to [64,8]
                    # (fast reciprocal: 8 elems/lane, not 512 on one lane)
                    # + PE broadcast. Both chains ride scalar — it is idle
                    # after the last exp, while sync still drains norm DMAs
                    # and gpsimd head-of-line-blocks on yg loads waiting for
                    # earlier collectives.
                    eng = nc.scalar
                    rf = ypool.tile([DH, NT // DH], F32,
                                    name=f"rtf{h}", tag="rf")
                    eng.dma_start(rf[:], yc[DH:DH + 1, :])
                    rfi = ypool.tile([DH, NT // DH], F32,
                                     name=f"rtfi{h}", tag="rfi")
                    nc.vector.reciprocal(rfi[:], rf[:])
                    eng.dma_start(yc[DH:DH + 1, :], rfi[:])
                    bc = y_ps.tile([DH, NT], F32, name=f"bc{h}", tag="yps")
                    nc.tensor.matmul(
                        bc[:], ones_bc[DH:DH + 1, :], yc[DH:DH + 1, :],
                        start=True, stop=True,
                    )
                    nc.vector.tensor_mul(
                        y_sb[h * DH:(h + 1) * DH, :], yc[0:DH, :], bc[:])
                elif (b, tt) == (B - 1, 2):
                    # tile 2 of batch 3: its exchange gates the final CC's
                    # pipe slot. SBUF->SBUF fold (no DRAM bounce) on gpsimd
                    # (free of exp traffic, and its only blocker — the yg
                    # loads — now ride sync); broadcast still via DRAM, no
                    # PE injection during tile-3 attention.
                    eng = nc.gpsimd
                    rf = ypool.tile([DH, NT // DH], F32,
                                    name=f"rf{b}{tt}{h}", tag="rf")
                    eng.dma_start(rf[:], yc[DH:DH + 1, :])
                    rfi = ypool.tile([DH, NT // DH], F32,
                                     name=f"rfi{b}{tt}{h}", tag="rfi")
                    nc.vector.reciprocal(rfi[:], rf[:])
                    ri_d = dram.tile([DH, NT // DH], F32, name=f"rid{b}{tt}{h}",
                                     tag=f"rid{b}{tt}{h}")
                    eng.dma_start(ri_d[:], rfi[:])
                    rbi = ypool.tile([DH, NT], F32, name=f"ri{b}{tt}{h}", tag="rbi")
                    eng.dma_start(
                        rbi[:],
                        bass.AP(tensor=ri_d.tensor, offset=ri_d.offset,
                                ap=[[0, DH], [1, NT]]),
                    )
                    nc.vector.tensor_mul(
                        y_sb[h * DH:(h + 1) * DH, :], yc[0:DH, :], rbi[:])
                else:
                    r_d = dram.tile([1, NT], F32, name=f"rd{b}{tt}{h}", tag=f"rd{b}{tt}{h}")
                    nc.sync.dma_start(r_d[:], yc[DH:DH + 1, :])
                    # fold r to [64, 8] so the reciprocal is free-size-8 on DVE
                    rf = ypool.tile([DH, NT // DH], F32, name=f"rf{b}{tt}{h}", tag="rf")
                    nc.sync.dma_start(
                        rf[:], r_d.rearrange("one (p f) -> (one p) f", p=DH))
                    rfi = ypool.tile([DH, NT // DH], F32, name=f"rfi{b}{tt}{h}", tag="rfi")
                    nc.vector.reciprocal(rfi[:], rf[:])
                    ri_d = dram.tile([DH, NT // DH], F32, name=f"rid{b}{tt}{h}",
                                     tag=f"rid{b}{tt}{h}")
                    nc.sync.dma_start(ri_d[:], rfi[:])
                    rbi = ypool.tile([DH, NT], F32, name=f"ri{b}{tt}{h}", tag="rbi")
                    nc.sync.dma_start(
                        rbi[:],
                        bass.AP(tensor=ri_d.tensor, offset=ri_d.offset,
                                ap=[[0, DH], [1, NT]]),
                    )
                    nc.vector.tensor_mul(
                        y_sb[h * DH:(h + 1) * DH, :], yc[0:DH, :], rbi[:])
                if h == 1:
                    # route this tile's half/per-tile slots to the owners'
                    # exchange buffers
                    if b < 3:
                        y_d = a2a_bufs[(b, 0)][0]
                        for half in range(2):
                            nc.sync.dma_start(
                                y_d[2 * tt + half],
                                y_sb[:, half * HT:(half + 1) * HT])
                        y_d_written[b] += 1
                        if y_d_written[b] == 4:
                            a2a_done[(b, 0)] = proj_a2a(b, 0)
                    elif tt < 2:
                        # group 0: 128-token quarter slots across tiles 0+1
                        y_d = a2a_bufs[(3, 0)][0]
                        ydr = y_d.rearrange("r p t -> p r t")
                        ysr = y_sb.rearrange("p (r t) -> p r t", r=4)
                        nc.sync.dma_start(
                            ydr[:, 4 * tt:4 * tt + 2, :], ysr[:, 0:2, :])
                        nc.gpsimd.dma_start(
                            ydr[:, 4 * tt + 2:4 * tt + 4, :], ysr[:, 2:4, :])
                        y_d_written[3] += 1
                        if y_d_written[3] == 2:
                            a2a_done[(3, 0)] = proj_a2a(3, 0)
                    else:
                        # groups 1/2: single tile, 64-token slots; writes on
                        # scalar+sync — emitted before the proj out DMAs so
                        # they sit ahead of them in-queue; gpsimd would
                        # head-of-line-block them behind yg-load CC waits
                        y_d = a2a_bufs[(3, tt - 1)][0]
                        ydr = y_d.rearrange("r p t -> p r t")
                        ysr = y_sb.rearrange("p (r t) -> p r t", r=NC)
                        nc.scalar.dma_start(
                            ydr[:, 0:NC // 2, :], ysr[:, 0:NC // 2, :])
                        nc.sync.dma_start(
                            ydr[:, NC // 2:, :], ysr[:, NC // 2:, :])
                        a2a_done[(3, tt - 1)] = proj_a2a(3, tt - 1)

            # ------------- pipelined main loop -----------------------------
            qkv_state = {}

            def _get_x(b, nt):
                if (b, nt) not in x_cache:
                    _load_x(b, nt)
                return x_cache[(b, nt)]

            def qkv_wn(b, nt, wn):
                if b not in qkv_state:
                    qkv_state[b] = (
                        qkvp.tile([128, T], BF16, name=f"q_{b}", tag="q"),
                        # zero-padded per-head k: slot h = [0..h*64) zeros,
                        # k_h rows, zeros — K=128 score matmuls
                        qkvp.tile([128, HPC, T], BF16, name=f"k_{b}", tag="k"),
                        qkvp.tile([128, T], BF16, name=f"v2T_{b}", tag="v2T"),
                        # v layout: [s-part, s-chunk, head, 72] fp8 — 65 used
                        # (64 dims + ones col), 72 so the DoubleRow k-subtile
                        # stride (2*72=144B) is 16B-aligned
                        qkvp.tile([128, T // 128, HPC, 72], FP8,
                                  name=f"v_{b}", tag="v"),
                    )
                    _, k_pad, _, _ = qkv_state[b]
                    nc.vector.memset(k_pad[DH:128, 0, :], 0.0)
                    nc.vector.memset(k_pad[0:DH, 1, :], 0.0)
                q_sb, k_pad, v2T, v_sb = qkv_state[b]
                x_sb = _get_x(b, nt)
                ps = mm_ps.tile([128, NT], F32, name=f"ps_{wn}{b}{nt}", tag="mm")
                for kp in range(KC // 2):
                    nc.tensor.matmul(
                        ps[:], signs[wn][:, 2 * kp:2 * kp + 2, :],
                        x_sb[:, 2 * kp:2 * kp + 2, :],
                        start=(kp == 0), stop=(kp == KC // 2 - 1),
                        perf_mode=DR,
                    )
                n0 = nt * NT
                if wn == "k":
                    for h in range(HPC):
                        nc.vector.tensor_scalar(
                            out=k_pad[h * DH:(h + 1) * DH, h, n0:n0 + NT],
                            in0=ps[h * DH:(h + 1) * DH, :],
                            scalar1=alpha_ap(wn, h * DH, (h + 1) * DH),
                            scalar2=bias_ap(wn, h * DH, (h + 1) * DH),
                            op0=mybir.AluOpType.mult, op1=mybir.AluOpType.add,
                        )
                else:
                    dst = q_sb if wn == "q" else v2T
                    nc.vector.tensor_scalar(
                        out=dst[:, n0:n0 + NT], in0=ps[:],
                        scalar1=alpha_ap(wn), scalar2=bias_ap(wn),
                        op0=mybir.AluOpType.mult, op1=mybir.AluOpType.add,
                    )
                if wn == "v":
                    x_cache.pop((b, nt), None)

            def qkv_vtrans(b, nt):
                # transpose v2T [o, s] chunks into av layout [s, (h, d)]
                q_sb, k_pad, v2T, v_sb = qkv_state[b]
                for ns in range(NT // 128):
                    sc_i = nt * (NT // 128) + ns
                    pst = y_ps.tile([128, 128], BF16, name=f"pst{b}{nt}{ns}", tag="yps")
                    nc.tensor.transpose(
                        pst[:], v2T[:, sc_i * 128:(sc_i + 1) * 128], ident[:]
                    )
                    nc.vector.tensor_copy(
                        out=v_sb[:, sc_i, :, 0:DH],
                        in_=pst.rearrange("p (h d) -> p h d", h=HPC),
                    )
                    nc.vector.memset(v_sb[:, sc_i, :, DH:DH + 1], 1.0)

            def attention_tt(b, tt, fill=(), mid_fill=(), k_groups=()):
                fill = list(fill)
                mid_fill = list(mid_fill)
                k_groups = list(k_groups)
                q_sb, k_pad, v2T, v_sb = qkv_state[b]
                t0 = tt * NT
                atts = []

                def scores_sc(sc):
                    s0 = sc * 128
                    pss = sc_ps.tile([128, HPC, NT], F32, name=f"s{b}{tt}{sc}", tag="sps")
                    for h in range(HPC):
                        nc.tensor.matmul(
                            pss[:, h, :], k_pad[:, h, s0:s0 + 128],
                            q_sb[:, t0:t0 + NT], start=True, stop=True,
                        )
                    # exp -> fp8 att, written per sc-pair tile so AV can run
                    # K=256 DoubleRow matmuls
                    if sc % 2 == 0:
                        atts.append(attp.tile([128, HPC, 2, NT], FP8,
                                              name=f"a{b}{tt}{sc}", tag="att"))
                    att2 = atts[sc // 2]
                    nc.scalar.activation(
                        out=att2[:, :, sc % 2, :], in_=pss[:],
                        func=mybir.ActivationFunctionType.Exp, scale=SCALE,
                    )

                def av_sc(jp):
                    att2 = atts[jp]
                    for h, psy in ((0, psA), (1, psB)):
                        nc.tensor.matmul(
                            psy[:], v_sb[:, 2 * jp:2 * jp + 2, h, 0:DH + 1],
                            att2[:, h, :, :],
                            start=(jp == 0), stop=(jp == T // 256 - 1),
                            perf_mode=DR,
                        )

                if mid_fill:
                    # head special: k-group emission interleaved with the
                    # score chunks it unblocks, then the v/q pipeline work,
                    # then AV — the PE never queues behind producers whose
                    # output the scores don't need yet
                    for sc in range(T // 128):
                        if sc % (NT // 128) == 0 and k_groups:
                            k_groups.pop(0)()
                        scores_sc(sc)
                    while mid_fill:
                        mid_fill.pop(0)()
                    psA = y_ps.tile([DH + 1, NT], F32, name=f"yA{b}{tt}", tag="yps")
                    psB = y_ps.tile([DH + 1, NT], F32, name=f"yB{b}{tt}", tag="yps")
                    for jp in range(T // 256):
                        av_sc(jp)
                else:
                    psA = y_ps.tile([DH + 1, NT], F32, name=f"yA{b}{tt}", tag="yps")
                    psB = y_ps.tile([DH + 1, NT], F32, name=f"yB{b}{tt}", tag="yps")
                    for sc in range(T // 128):
                        scores_sc(sc)
                        if sc % 2 == 1:
                            av_sc(sc // 2)
                y_sb = ysbp.tile([128, NT], BF16, name=f"ysb{b}{tt}", tag="ysb")
                for h, psy in ((0, psA), (1, psB)):
                    # one fast 65-lane copy releases the PSUM slot; the whole
                    # normalization chain runs from SBUF off the PE critical
                    # path (emitted one tt later).
                    yc = ypool.tile([DH + 1, NT], F32, name=f"yc{b}{tt}{h}", tag="yc")
                    nc.vector.tensor_copy(yc[:], psy[:])
                    pend_norm.append((b, tt, h, t0, yc, y_sb))
                while fill:
                    fill.pop(0)()
                while pend_norm:
                    emit_norm(pend_norm.pop(0))

            # batch-0 fill order: attention(0, tt0) needs q(nt0) + all k + v
            # chunks; emit the not-yet-needed q(nt1..3) after k so scores can
            # start earlier.
            qkv_wn(0, 0, "q")
            kgs = [lambda nn=nt: qkv_wn(0, nn, "k") for nt in range(T // NT)]
            mid = []
            for nt in range(1, T // NT):
                mid.append(lambda nn=nt: qkv_wn(0, nn, "q"))
            for nt in range(T // NT):
                mid.append(lambda nn=nt: qkv_wn(0, nn, "v"))
                mid.append(lambda nn=nt: qkv_vtrans(0, nn))
            for b in range(B):
                for tt in range(T // NT):
                    fills = []
                    if b + 1 < B:
                        fills += [
                            (lambda bb=b + 1, nn=tt, w=w: qkv_wn(bb, nn, w))
                            for w in ("q", "k", "v")
                        ]
                        fills.append(lambda bb=b + 1, nn=tt: qkv_vtrans(bb, nn))
                    # proj matmuls deferred ~2 slots past their AllToAll so
                    # the in-order PE queue never waits on the exchange
                    if b in (1, 2) and tt == 2:
                        fills.append(
                            lambda bb=b - 1: proj_mm(bb, 0, a2a_done[(bb, 0)]))
                    if (b, tt) == (0, 0):
                        attention_tt(b, tt, fills, mid_fill=mid, k_groups=kgs)
                    else:
                        attention_tt(b, tt, fills)
            while pend_norm:
                emit_norm(pend_norm.pop(0))
            # drain deferred proj work
            proj_mm(2, 0, a2a_done[(2, 0)])
            proj_mm(3, 0, a2a_done[(3, 0)])
            proj_mm(3, 1, a2a_done[(3, 1)])
            proj_mm(3, 2, a2a_done[(3, 2)])

    nc.finalize()
    return nc


def _host_prep(x, Wq, bq, Wk, bk, Wv, bv, Wp, bp):
    # contiguous per-tile layout [tile, c%128, c//128, token] so each x tile
    # is one 4KB-per-partition DMA descriptor chain instead of 1024x512B;
    # x and the qkv signs are fp8e4 (signs exact, x ~2% quantization) so
    # QKV matmuls run K=256 DoubleRow
    xt = x.reshape(NTOK, C).T.astype(ml_dtypes.float8_e4m3)  # [1024, 8192]
    xh = np.ascontiguousarray(
        xt.reshape(KC, 128, NTOK // NT, NT).transpose(2, 1, 0, 3))

    def sign_t(Wsl):
        # sign(W[sl]) [128, 1024] -> wT-layout [128(part=c%128), KC, 128(o)]
        s = np.sign(Wsl).T.astype(ml_dtypes.float8_e4m3)       # [1024, 128]
        return np.ascontiguousarray(
            s.reshape(KC, 128, OS).transpose(1, 0, 2))         # [128, KC, OS]

    # full proj weight, every core identical:
    # sp[p, r, m, o'] = sign(Wp[m*128+o', r*128+p])
    spT = np.sign(Wp).T.astype(ml_dtypes.bfloat16)             # [1024 c, 1024 o]
    sp_full = np.ascontiguousarray(
        spT.reshape(KC, 128, KC, 128).transpose(1, 0, 2, 3))
    ap_full = np.ascontiguousarray(
        np.abs(Wp).mean(1).astype(np.float32).reshape(KC, 128).T)
    bp_full = np.ascontiguousarray(bp.astype(np.float32).reshape(KC, 128).T)

    in_maps = []
    for i in range(NC):
        sl = slice(OS * i, OS * (i + 1))
        m = {
            "xH": xh,
            "sq": sign_t(Wq[sl]),
            "sk": sign_t(Wk[sl]),
            "sv": sign_t(Wv[sl]),
            "sp": sp_full,
            "ab": np.ascontiguousarray(np.stack([
                np.abs(Wq[sl]).mean(1), np.abs(Wk[sl]).mean(1),
                np.abs(Wv[sl]).mean(1), bq[sl], bk[sl], bv[sl],
            ], axis=1).astype(np.float32)),
            "apf": ap_full,
            "bpf": bp_full,
        }
        in_maps.append(m)
    return in_maps


def kernel(x, Wq, bq, Wk, bk, Wv, bv, Wp, bp, _trace=False, _trace_cores=None):
    if "nc" not in _CACHED:
        _CACHED["nc"] = _build()
    nc = _CACHED["nc"]
    in_maps = _host_prep(x, Wq, bq, Wk, bk, Wv, bv, Wp, bp)
    res = run_bass_kernel_spmd(
        nc, in_maps, core_ids=list(range(NC)),
        trace=_trace, trace_cores=_trace_cores,
    )
    _CACHED["last_results"] = res
    # out_t per core: [B, 1024, 256]. Batches 0..2: core i owns half-tile
    # tokens (i//2)*512 + (i%2)*256. Batch 3: four 64-token slots, one per
    # 512-token tile: tokens tt*512 + i*64.
    out = np.empty((B, T, C), np.float32)
    for i in range(NC):
        o = res.results[i]["out_t"]          # [B, C, HT]
        for b in range(B - 1):
            t0 = (i // 2) * NT + (i % 2) * HT
            out[b, t0:t0 + HT, :] = o[b].T
        # batch 3: group 0 = tiles 0+1 (128-token slots), groups 1/2 =
        # tiles 2/3 (64-token slots)
        out[3, i * 128:(i + 1) * 128, :] = o[3, :, 0:128].T
        out[3, 1024 + i * 64:1024 + (i + 1) * 64, :] = o[3, :, 128:192].T
        out[3, 1536 + i * 64:1536 + (i + 1) * 64, :] = o[3, :, 192:256].T
    return out



# revision 77
# speedup vs baseline: 1.0173x; 1.0173x over previous
"""BinaryAttention on 8 TRN2 NeuronCores (Bass/Tile, SPMD tensor-parallel).

Math (per reference):
  Wb = alpha * sign(W), alpha[o] = mean_c |W[o,c]|
  q/k/v = x @ Wb_{q,k,v}^T + b;   att = softmax(q k^T / sqrt(Dh));
  y = att @ v;  out = y @ Wb_p^T + bp

Sharding (8 cores):
  - Heads (16) sharded 2/core: each core computes q,k,v for its 2 heads over
    all (B,T) and runs attention for them, producing its 128 y-channels.
  - Proj is token-sharded: core i owns a 256-token half-tile per batch. One
    AllToAll per batch routes each y element to exactly its owner (2MB/core
    total instead of a 16MB/core AllGather replication), then each core
    projects its own tokens against the full binarized Wp.

Score matmuls run with K=128 by keeping k in a zero-padded per-head layout
(k_pad[:, h] has k_h in rows h*64..h*64+63, zeros elsewhere): the PE streams
~2x faster at K>=96 than at K=64, and the zero rows annihilate the other
head's q rows. q/k/scores stay bf16; x, the qkv sign weights, exp(att) and
v are fp8e4 so the QKV and AV matmuls run K=256 fp8 DoubleRow (~2x PE
throughput; rel err ~1.6e-2 vs the 2e-2 budget). alpha/bias applied in fp32
on PSUM. Softmax skips max-subtraction (scores are O(1) here). Binarization
(sign/alpha) is precomputed on the host.
"""

import numpy as np
import ml_dtypes

import concourse.bass as bass
import concourse.bacc as bacc
import concourse.tile as tile
from concourse import mybir
from concourse.masks import make_identity
from concourse.bass_utils import run_bass_kernel_spmd

NC = 8          # cores
B, T, C = 4, 2048, 1024
H, DH = 16, 64
HPC = H // NC   # heads per core = 2
OS = HPC * DH   # per-core o-slice width = 128
KC = C // 128   # contraction chunks = 8
NTOK = B * T    # 8192
NT = 512        # moving-operand tile (fp32 psum bank)
HT = NT // 2    # half-tile tokens owned per core per batch = 256
SCALE = DH ** -0.5

F32 = mybir.dt.float32
BF16 = mybir.dt.bfloat16
FP8 = mybir.dt.float8e4
DR = mybir.MatmulPerfMode.DoubleRow

_CACHED = {}


def _build():
    nc = bacc.Bacc("TRN2", target_bir_lowering=False, debug=False, num_devices=NC)

    xH = nc.dram_tensor("xH", [NTOK // NT, 128, KC, NT], FP8,
                        kind="ExternalInput")
    sq_d = nc.dram_tensor("sq", [128, KC, OS], FP8, kind="ExternalInput")
    sk_d = nc.dram_tensor("sk", [128, KC, OS], FP8, kind="ExternalInput")
    sv_d = nc.dram_tensor("sv", [128, KC, OS], FP8, kind="ExternalInput")
    # full binarized proj weight: [128(c%128), c-block, o-block, 128(o%128)]
    sp_d = nc.dram_tensor("sp", [128, KC, KC, 128], BF16, kind="ExternalInput")
    ab_d = nc.dram_tensor("ab", [128, 6], F32, kind="ExternalInput")
    ap_d = nc.dram_tensor("apf", [128, KC], F32, kind="ExternalInput")
    bp_d = nc.dram_tensor("bpf", [128, KC], F32, kind="ExternalInput")
    # token-sharded output: per batch, this core's 256 tokens x all 1024 outs.
    # Batches 0..2: one 256-token half-tile (tt=i//2, half=i%2). Batch 3 is
    # exchanged as two quarter-tile AllToAlls (shorter tail): this core owns
    # 128-token quarter i%4 of tile i//4 in each half of the batch.
    out_t = nc.dram_tensor("out_t", [B, C, HT], F32, kind="ExternalOutput")

    with tile.TileContext(nc, num_cores=NC) as tc:
        with (
            tc.tile_pool(name="const", bufs=1) as const,
            tc.tile_pool(name="xin", bufs=5) as xin,
            tc.tile_pool(name="qkv", bufs=2) as qkvp,
            tc.tile_pool(name="attp", bufs=9) as attp,
            tc.tile_pool(name="ypool", bufs=8) as ypool,
            tc.tile_pool(name="ysb", bufs=4) as ysbp,
            tc.tile_pool(name="ygp", bufs=3) as ygp,
            tc.tile_pool(name="outp", bufs=2) as outp,
            tc.tile_pool(name="mm_ps", bufs=2, space="PSUM") as mm_ps,
            tc.tile_pool(name="sc_ps", bufs=2, space="PSUM") as sc_ps,
            tc.tile_pool(name="y_ps", bufs=2, space="PSUM") as y_ps,
            tc.tile_pool(name="dram", bufs=1, space="DRAM") as dram,
        ):
            # ---------------- prologue: load host-binarized weights --------
            # startup is DMA-latency-bound: interleave sq and x(0,0) in
            # single-kc chunks round-robin over 4 queues so QKV matmul kc=0
            # can fire as soon as ~160KB lands instead of ~770KB
            signs = {}
            for wn, s_d in (("q", sq_d), ("k", sk_d), ("v", sv_d)):
                s_sb = const.tile([128, KC, OS], FP8, name=f"s_{wn}", tag=f"s_{wn}")
                signs[wn] = s_sb
            ab_sb = const.tile([128, 6], F32, tag="ab")
            nc.scalar.dma_start(ab_sb[:], ab_d[:])
            _abcol = {"q": 0, "k": 1, "v": 2}

            def alpha_ap(w, p0=0, p1=128):
                c = _abcol[w]
                return ab_sb[p0:p1, c:c + 1]

            def bias_ap(w, p0=0, p1=128):
                c = 3 + _abcol[w]
                return ab_sb[p0:p1, c:c + 1]

            ident = const.tile([128, 128], BF16, tag="ident")
            make_identity(nc, ident)

            x_cache = {}
            _q4 = [nc.sync, nc.gpsimd, nc.scalar]

            def _load_x(b, nt):
                x_sb = xin.tile([128, KC, NT], FP8, name=f"x_{b}_{nt}", tag="x")
                ti = b * (T // NT) + nt
                if (b, nt) == (0, 0):
                    # emitted inline with the sq chunks below
                    pass
                elif b == 0:
                    # half-tile per queue: first x usable in ~7us not ~13
                    nc.sync.dma_start(
                        x_sb[:, 0:KC // 2, :], xH[ti, :, 0:KC // 2, :])
                    nc.gpsimd.dma_start(
                        x_sb[:, KC // 2:, :], xH[ti, :, KC // 2:, :])
                else:
                    q = nc.sync if nt % 2 == 0 else nc.scalar
                    q.dma_start(x_sb[:], xH[ti])
                x_cache[(b, nt)] = x_sb
                return x_sb

            x00 = _load_x(0, 0)
            for kc in range(KC):
                eng = _q4[kc % 3]
                eng.dma_start(
                    signs["q"][:, kc:kc + 1, :], sq_d[:, kc:kc + 1, :])
                eng.dma_start(x00[:, kc:kc + 1, :], xH[0, :, kc:kc + 1, :])
            # k signs next (needed ~2nd in the QKV pipeline), split 3-way
            for i, (c0, c1) in enumerate(((0, 3), (3, 6), (6, 8))):
                _q4[i].dma_start(
                    signs["k"][:, c0:c1, :], sk_d[:, c0:c1, :])
            # remaining batch-0 tiles in kc-thirds across all three queues
            # so k(0,1..3) aren't starved behind two-queue half-tile loads
            for nt in range(1, T // NT):
                x_sb = xin.tile([128, KC, NT], FP8, name=f"x_0_{nt}", tag="x")
                for i, (c0, c1) in enumerate(((0, 3), (3, 6), (6, 8))):
                    _q4[(nt + i) % 3].dma_start(
                        x_sb[:, c0:c1, :], xH[nt, :, c0:c1, :])
                x_cache[(0, nt)] = x_sb
            nc.scalar.dma_start(
                signs["v"][:, 0:KC // 2, :], sv_d[:, 0:KC // 2, :])
            nc.scalar.dma_start(
                signs["v"][:, KC // 2:, :], sv_d[:, KC // 2:, :])

            # tiny warmup collective: the first CC op pays ~25us of ring
            # setup; spend it here while the PE chews on batch-0 QKV
            wu_in = dram.tile([NC, 8], BF16, name="wu_i", tag="wu_i")
            wu_out = dram.tile([NC, 8], BF16, name="wu_o", tag="wu_o")
            wu_sb = const.tile([1, NC * 8], BF16, tag="wu")
            nc.vector.memset(wu_sb[:], 0.0)
            nc.gpsimd.dma_start(wu_in.rearrange("r t -> (r t)")[None, :], wu_sb[:])
            nc.gpsimd.collective_compute(
                "AllToAll", mybir.AluOpType.bypass,
                replica_groups=[list(range(NC))],
                ins=[wu_in.opt()], outs=[wu_out.opt()],
            )

            # proj weights arrive behind the warmup on the gpsimd queue
            # (not needed until ~150us); keeps the sync queue free for x
            sp_sb = const.tile([128, KC, KC, 128], BF16, tag="s_p")
            nc.scalar.dma_start(sp_sb[:], sp_d[:])
            ap_sb = const.tile([128, KC], F32, tag="a_p")
            nc.scalar.dma_start(ap_sb[:], ap_d[:])
            bp_sb = const.tile([128, KC], F32, tag="b_p")
            nc.scalar.dma_start(bp_sb[:], bp_d[:])

            # ------------- token-sharded proj via AllToAll -----------------
            # groups: (b, 0) for b<3 are full batches (half-tile blocks of
            # 256 tokens); batch 3 exchanges per-tile (64-token slots) so
            # the final, unhidable exchange is as small as possible.
            a2a_bufs = {}
            for b in range(B - 1):
                y_d = dram.tile([NC, 128, HT], BF16, name=f"yd{b}", tag=f"yd{b}")
                yg_d = dram.tile([NC, 128, HT], BF16, name=f"ygd{b}", tag=f"ygd{b}")
                a2a_bufs[(b, 0)] = (y_d, yg_d, HT)
            # batch 3: tiles 0+1 in one group, then tiles 2 and 3 alone so
            # the CC pipe is idle when the final (smallest) exchange fires
            for g, w3 in ((0, HT // 2), (1, NT // NC), (2, NT // NC)):
                y_d = dram.tile([NC, 128, w3], BF16, name=f"yd3{g}",
                                tag=f"yd3{g}")
                yg_d = dram.tile([NC, 128, w3], BF16, name=f"ygd3{g}",
                                 tag=f"ygd3{g}")
                a2a_bufs[(3, g)] = (y_d, yg_d, w3)
            y_d_written = {b: 0 for b in range(B)}

            def proj_a2a(b, g, load=True):
                y_d, yg_d, w = a2a_bufs[(b, g)]
                nc.gpsimd.collective_compute(
                    "AllToAll", mybir.AluOpType.bypass,
                    replica_groups=[list(range(NC))],
                    ins=[y_d.opt()], outs=[yg_d.opt()],
                )
                if not load:
                    return None
                yg_sb = ygp.tile([128, NC, w], BF16, name=f"yg{b}{g}", tag="yg")
                nc.gpsimd.dma_start(
                    yg_sb[:], yg_d.rearrange("r p t -> p r t"))
                return yg_sb

            def proj_mm(b, g, yg_sb):
                _, _, w = a2a_bufs[(b, g)]
                t0 = 0 if b < 3 else (0, HT // 2, HT // 2 + NT // NC)[g]
                orr = out_t[b, :, t0:t0 + w].rearrange("(m p) t -> p m t", p=128)
                o_sb = outp.tile([128, KC, w], F32, name=f"ob{b}{g}", tag="ob")
                for m in range(KC):
                    pp = mm_ps.tile([128, w], F32, name=f"pp{b}{g}{m}", tag="mm")
                    for r in range(NC):
                        nc.tensor.matmul(
                            pp[:], sp_sb[:, r, m, :], yg_sb[:, r, :],
                            start=(r == 0), stop=(r == NC - 1),
                        )
                    nc.vector.tensor_scalar(
                        out=o_sb[:, m, :], in0=pp[:],
                        scalar1=ap_sb[:, m:m + 1], scalar2=bp_sb[:, m:m + 1],
                        op0=mybir.AluOpType.mult, op1=mybir.AluOpType.add,
                    )
                if (b, g) == (3, 2):
                    nc.gpsimd.dma_start(orr[:, 0:KC // 2], o_sb[:, 0:KC // 2])
                    nc.sync.dma_start(orr[:, KC // 2:], o_sb[:, KC // 2:])
                else:
                    # out DMAs ride sync: scalar must be free at the tail for
                    # the last tile's norm chain
                    nc.sync.dma_start(orr, o_sb[:])

            a2a_done = {}

            pend_norm = []

            # all-ones stationary for the tail-norm PE broadcast (only the
            # partition-64 row is read)
            ones_bc = const.tile([DH + 1, DH], F32, tag="ones_bc")
            nc.vector.memset(ones_bc[:], 1.0)

            def emit_norm(item):
                b, tt, h, t0, yc, y_sb = item
                if (b, tt) == (B - 1, T // NT - 1):
                    # last tile: the whole norm chain is on the critical tail
                    # before the final AllToAll. SBUF->SBUF fold to [64,8]
                    # (fast reciprocal: 8 elems/lane, not 512 on one lane)
                    # + PE broadcast. Both chains ride scalar — it is idle
                    # after the last exp, while sync still drains norm DMAs
                    # and gpsimd head-of-line-blocks on yg loads waiting for
                    # earlier collectives.
                    eng = nc.scalar
                    rf = ypool.tile([DH, NT // DH], F32,
                                    name=f"rtf{h}", tag="rf")
                    eng.dma_start(rf[:], yc[DH:DH + 1, :])
                    rfi = ypool.tile([DH, NT // DH], F32,
                                     name=f"rtfi{h}", tag="rfi")
                    nc.vector.reciprocal(rfi[:], rf[:])
                    eng.dma_start(yc[DH:DH + 1, :], rfi[:])
                    bc = y_ps.tile([DH, NT], F32, name=f"bc{h}", tag="yps")
                    nc.tensor.matmul(
                        bc[:], ones_bc[DH:DH + 1, :], yc[DH:DH + 1, :],
                        start=True, stop=True,
                    )
                    nc.vector.tensor_mul(
                        y_sb[h * DH:(h + 1) * DH, :], yc[0:DH, :], bc[:])
                else:
                    r_d = dram.tile([1, NT], F32, name=f"rd{b}{tt}{h}", tag=f"rd{b}{tt}{h}")
                    nc.sync.dma_start(r_d[:], yc[DH:DH + 1, :])
                    # fold r to [64, 8] so the reciprocal is free-size-8 on DVE
                    rf = ypool.tile([DH, NT // DH], F32, name=f"rf{b}{tt}{h}", tag="rf")
                    nc.sync.dma_start(
                        rf[:], r_d.rearrange("one (p f) -> (one p) f", p=DH))
                    rfi = ypool.tile([DH, NT // DH], F32, name=f"rfi{b}{tt}{h}", tag="rfi")
                    nc.vector.reciprocal(rfi[:], rf[:])
                    ri_d = dram.tile([DH, NT // DH], F32, name=f"rid{b}{tt}{h}",
                                     tag=f"rid{b}{tt}{h}")
                    nc.sync.dma_start(ri_d[:], rfi[:])
                    rbi = ypool.tile([DH, NT], F32, name=f"ri{b}{tt}{h}", tag="rbi")
                    nc.sync.dma_start(
                        rbi[:],
                        bass.AP(tensor=ri_d.tensor, offset=ri_d.offset,
                                ap=[[0, DH], [1, NT]]),
                    )
                    nc.vector.tensor_mul(
                        y_sb[h * DH:(h + 1) * DH, :], yc[0:DH, :], rbi[:])
                if h == 1:
                    # route this tile's half/per-tile slots to the owners'
                    # exchange buffers
                    if b < 3:
                        y_d = a2a_bufs[(b, 0)][0]
                        for half in range(2):
                            nc.sync.dma_start(
                                y_d[2 * tt + half],
                                y_sb[:, half * HT:(half + 1) * HT])
                        y_d_written[b] += 1
                        if y_d_written[b] == 4:
                            a2a_done[(b, 0)] = proj_a2a(b, 0)
                    elif tt < 2:
                        # group 0: 128-token quarter slots across tiles 0+1
                        y_d = a2a_bufs[(3, 0)][0]
                        ydr = y_d.rearrange("r p t -> p r t")
                        ysr = y_sb.rearrange("p (r t) -> p r t", r=4)
                        nc.sync.dma_start(
                            ydr[:, 4 * tt:4 * tt + 2, :], ysr[:, 0:2, :])
                        nc.gpsimd.dma_start(
                            ydr[:, 4 * tt + 2:4 * tt + 4, :], ysr[:, 2:4, :])
                        y_d_written[3] += 1
                        if y_d_written[3] == 2:
                            a2a_done[(3, 0)] = proj_a2a(3, 0)
                    else:
                        # groups 1/2: single tile, 64-token slots; writes on
                        # scalar+sync — emitted before the proj out DMAs so
                        # they sit ahead of them in-queue; gpsimd would
                        # head-of-line-block them behind yg-load CC waits
                        y_d = a2a_bufs[(3, tt - 1)][0]
                        ydr = y_d.rearrange("r p t -> p r t")
                        ysr = y_sb.rearrange("p (r t) -> p r t", r=NC)
                        nc.scalar.dma_start(
                            ydr[:, 0:NC // 2, :], ysr[:, 0:NC // 2, :])
                        nc.sync.dma_start(
                            ydr[:, NC // 2:, :], ysr[:, NC // 2:, :])
                        a2a_done[(3, tt - 1)] = proj_a2a(3, tt - 1)

            # ------------- pipelined main loop -----------------------------
            qkv_state = {}

            def _get_x(b, nt):
                if (b, nt) not in x_cache:
                    _load_x(b, nt)
                return x_cache[(b, nt)]

            def qkv_wn(b, nt, wn):
                if b not in qkv_state:
                    qkv_state[b] = (
                        qkvp.tile([128, T], BF16, name=f"q_{b}", tag="q"),
                        # zero-padded per-head k: slot h = [0..h*64) zeros,
                        # k_h rows, zeros — K=128 score matmuls
                        qkvp.tile([128, HPC, T], BF16, name=f"k_{b}", tag="k"),
                        qkvp.tile([128, T], BF16, name=f"v2T_{b}", tag="v2T"),
                        # v layout: [s-part, s-chunk, head, 72] fp8 — 65 used
                        # (64 dims + ones col), 72 so the DoubleRow k-subtile
                        # stride (2*72=144B) is 16B-aligned
                        qkvp.tile([128, T // 128, HPC, 72], FP8,
                                  name=f"v_{b}", tag="v"),
                    )
                    _, k_pad, _, _ = qkv_state[b]
                    nc.vector.memset(k_pad[DH:128, 0, :], 0.0)
                    nc.vector.memset(k_pad[0:DH, 1, :], 0.0)
                q_sb, k_pad, v2T, v_sb = qkv_state[b]
                x_sb = _get_x(b, nt)
                ps = mm_ps.tile([128, NT], F32, name=f"ps_{wn}{b}{nt}", tag="mm")
                for kp in range(KC // 2):
                    nc.tensor.matmul(
                        ps[:], signs[wn][:, 2 * kp:2 * kp + 2, :],
                        x_sb[:, 2 * kp:2 * kp + 2, :],
                        start=(kp == 0), stop=(kp == KC // 2 - 1),
                        perf_mode=DR,
                    )
                n0 = nt * NT
                if wn == "k":
                    for h in range(HPC):
                        nc.vector.tensor_scalar(
                            out=k_pad[h * DH:(h + 1) * DH, h, n0:n0 + NT],
                            in0=ps[h * DH:(h + 1) * DH, :],
                            scalar1=alpha_ap(wn, h * DH, (h + 1) * DH),
                            scalar2=bias_ap(wn, h * DH, (h + 1) * DH),
                            op0=mybir.AluOpType.mult, op1=mybir.AluOpType.add,
                        )
                else:
                    dst = q_sb if wn == "q" else v2T
                    nc.vector.tensor_scalar(
                        out=dst[:, n0:n0 + NT], in0=ps[:],
                        scalar1=alpha_ap(wn), scalar2=bias_ap(wn),
                        op0=mybir.AluOpType.mult, op1=mybir.AluOpType.add,
                    )
                if wn == "v":
                    x_cache.pop((b, nt), None)

            def qkv_vtrans(b, nt):
                # transpose v2T [o, s] chunks into av layout [s, (h, d)]
                q_sb, k_pad, v2T, v_sb = qkv_state[b]
                for ns in range(NT // 128):
                    sc_i = nt * (NT // 128) + ns
                    pst = y_ps.tile([128, 128], BF16, name=f"pst{b}{nt}{ns}", tag="yps")
                    nc.tensor.transpose(
                        pst[:], v2T[:, sc_i * 128:(sc_i + 1) * 128], ident[:]
                    )
                    nc.vector.tensor_copy(
                        out=v_sb[:, sc_i, :, 0:DH],
                        in_=pst.rearrange("p (h d) -> p h d", h=HPC),
                    )
                    nc.vector.memset(v_sb[:, sc_i, :, DH:DH + 1], 1.0)

            def attention_tt(b, tt, fill=(), mid_fill=(), k_groups=()):
                fill = list(fill)
                mid_fill = list(mid_fill)
                k_groups = list(k_groups)
                q_sb, k_pad, v2T, v_sb = qkv_state[b]
                t0 = tt * NT
                atts = []

                def scores_sc(sc):
                    s0 = sc * 128
                    pss = sc_ps.tile([128, HPC, NT], F32, name=f"s{b}{tt}{sc}", tag="sps")
                    for h in range(HPC):
                        nc.tensor.matmul(
                            pss[:, h, :], k_pad[:, h, s0:s0 + 128],
                            q_sb[:, t0:t0 + NT], start=True, stop=True,
                        )
                    # exp -> fp8 att, written per sc-pair tile so AV can run
                    # K=256 DoubleRow matmuls
                    if sc % 2 == 0:
                        atts.append(attp.tile([128, HPC, 2, NT], FP8,
                                              name=f"a{b}{tt}{sc}", tag="att"))
                    att2 = atts[sc // 2]
                    nc.scalar.activation(
                        out=att2[:, :, sc % 2, :], in_=pss[:],
                        func=mybir.ActivationFunctionType.Exp, scale=SCALE,
                    )

                def av_sc(jp):
                    att2 = atts[jp]
                    for h, psy in ((0, psA), (1, psB)):
                        nc.tensor.matmul(
                            psy[:], v_sb[:, 2 * jp:2 * jp + 2, h, 0:DH + 1],
                            att2[:, h, :, :],
                            start=(jp == 0), stop=(jp == T // 256 - 1),
                            perf_mode=DR,
                        )

                if mid_fill:
                    # head special: k-group emission interleaved with the
                    # score chunks it unblocks, then the v/q pipeline work,
                    # then AV — the PE never queues behind producers whose
                    # output the scores don't need yet
                    for sc in range(T // 128):
                        if sc % (NT // 128) == 0 and k_groups:
                            k_groups.pop(0)()
                        scores_sc(sc)
                    while mid_fill:
                        mid_fill.pop(0)()
                    psA = y_ps.tile([DH + 1, NT], F32, name=f"yA{b}{tt}", tag="yps")
                    psB = y_ps.tile([DH + 1, NT], F32, name=f"yB{b}{tt}", tag="yps")
                    for jp in range(T // 256):
                        av_sc(jp)
                else:
                    psA = y_ps.tile([DH + 1, NT], F32, name=f"yA{b}{tt}", tag="yps")
                    psB = y_ps.tile([DH + 1, NT], F32, name=f"yB{b}{tt}", tag="yps")
                    for sc in range(T // 128):
                        scores_sc(sc)
                        if sc % 2 == 1:
                            av_sc(sc // 2)
                y_sb = ysbp.tile([128, NT], BF16, name=f"ysb{b}{tt}", tag="ysb")
                for h, psy in ((0, psA), (1, psB)):
                    # one fast 65-lane copy releases the PSUM slot; the whole
                    # normalization chain runs from SBUF off the PE critical
                    # path (emitted one tt later).
                    yc = ypool.tile([DH + 1, NT], F32, name=f"yc{b}{tt}{h}", tag="yc")
                    nc.vector.tensor_copy(yc[:], psy[:])
                    pend_norm.append((b, tt, h, t0, yc, y_sb))
                while fill:
                    fill.pop(0)()
                while pend_norm:
                    emit_norm(pend_norm.pop(0))

            # batch-0 fill order: attention(0, tt0) needs q(nt0) + all k + v
            # chunks; emit the not-yet-needed q(nt1..3) after k so scores can
            # start earlier.
            qkv_wn(0, 0, "q")
            kgs = [lambda nn=nt: qkv_wn(0, nn, "k") for nt in range(T // NT)]
            mid = []
            for nt in range(1, T // NT):
                mid.append(lambda nn=nt: qkv_wn(0, nn, "q"))
            for nt in range(T // NT):
                mid.append(lambda nn=nt: qkv_wn(0, nn, "v"))
                mid.append(lambda nn=nt: qkv_vtrans(0, nn))
            for b in range(B):
                for tt in range(T // NT):
                    fills = []
                    if b + 1 < B:
                        fills += [
                            (lambda bb=b + 1, nn=tt, w=w: qkv_wn(bb, nn, w))
                            for w in ("q", "k", "v")
                        ]
                        fills.append(lambda bb=b + 1, nn=tt: qkv_vtrans(bb, nn))
                    # proj matmuls deferred ~2 slots past their AllToAll so
                    # the in-order PE queue never waits on the exchange
                    if b in (1, 2) and tt == 2:
                        fills.append(
                            lambda bb=b - 1: proj_mm(bb, 0, a2a_done[(bb, 0)]))
                    if (b, tt) == (0, 0):
                        attention_tt(b, tt, fills, mid_fill=mid, k_groups=kgs)
                    else:
                        attention_tt(b, tt, fills)
            while pend_norm:
                emit_norm(pend_norm.pop(0))
            # drain deferred proj work
            proj_mm(2, 0, a2a_done[(2, 0)])
            proj_mm(3, 0, a2a_done[(3, 0)])
            proj_mm(3, 1, a2a_done[(3, 1)])
            proj_mm(3, 2, a2a_done[(3, 2)])

    nc.finalize()
    return nc


def _host_prep(x, Wq, bq, Wk, bk, Wv, bv, Wp, bp):
    # contiguous per-tile layout [tile, c%128, c//128, token] so each x tile
    # is one 4KB-per-partition DMA descriptor chain instead of 1024x512B;
    # x and the qkv signs are fp8e4 (signs exact, x ~2% quantization) so
    # QKV matmuls run K=256 DoubleRow
    xt = x.reshape(NTOK, C).T.astype(ml_dtypes.float8_e4m3)  # [1024, 8192]
    xh = np.ascontiguousarray(
        xt.reshape(KC, 128, NTOK // NT, NT).transpose(2, 1, 0, 3))

    def sign_t(Wsl):
        # sign(W[sl]) [128, 1024] -> wT-layout [128(part=c%128), KC, 128(o)]
        s = np.sign(Wsl).T.astype(ml_dtypes.float8_e4m3)       # [1024, 128]
        return np.ascontiguousarray(
            s.reshape(KC, 128, OS).transpose(1, 0, 2))         # [128, KC, OS]

    # full proj weight, every core identical:
    # sp[p, r, m, o'] = sign(Wp[m*128+o', r*128+p])
    spT = np.sign(Wp).T.astype(ml_dtypes.bfloat16)             # [1024 c, 1024 o]
    sp_full = np.ascontiguousarray(
        spT.reshape(KC, 128, KC, 128).transpose(1, 0, 2, 3))
    ap_full = np.ascontiguousarray(
        np.abs(Wp).mean(1).astype(np.float32).reshape(KC, 128).T)
    bp_full = np.ascontiguousarray(bp.astype(np.float32).reshape(KC, 128).T)

    in_maps = []
    for i in range(NC):
        sl = slice(OS * i, OS * (i + 1))
        m = {
            "xH": xh,
            "sq": sign_t(Wq[sl]),
            "sk": sign_t(Wk[sl]),
            "sv": sign_t(Wv[sl]),
            "sp": sp_full,
            "ab": np.ascontiguousarray(np.stack([
                np.abs(Wq[sl]).mean(1), np.abs(Wk[sl]).mean(1),
                np.abs(Wv[sl]).mean(1), bq[sl], bk[sl], bv[sl],
            ], axis=1).astype(np.float32)),
            "apf": ap_full,
            "bpf": bp_full,
        }
        in_maps.append(m)
    return in_maps


def kernel(x, Wq, bq, Wk, bk, Wv, bv, Wp, bp, _trace=False, _trace_cores=None):
    if "nc" not in _CACHED:
        _CACHED["nc"] = _build()
    nc = _CACHED["nc"]
    in_maps = _host_prep(x, Wq, bq, Wk, bk, Wv, bv, Wp, bp)
    res = run_bass_kernel_spmd(
        nc, in_maps, core_ids=list(range(NC)),
        trace=_trace, trace_cores=_trace_cores,
    )
    _CACHED["last_results"] = res
    # out_t per core: [B, 1024, 256]. Batches 0..2: core i owns half-tile
    # tokens (i//2)*512 + (i%2)*256. Batch 3: four 64-token slots, one per
    # 512-token tile: tokens tt*512 + i*64.
    out = np.empty((B, T, C), np.float32)
    for i in range(NC):
        o = res.results[i]["out_t"]          # [B, C, HT]
        for b in range(B - 1):
            t0 = (i // 2) * NT + (i % 2) * HT
            out[b, t0:t0 + HT, :] = o[b].T
        # batch 3: group 0 = tiles 0+1 (128-token slots), groups 1/2 =
        # tiles 2/3 (64-token slots)
        out[3, i * 128:(i + 1) * 128, :] = o[3, :, 0:128].T
        out[3, 1024 + i * 64:1024 + (i + 1) * 64, :] = o[3, :, 128:192].T
        out[3, 1536 + i * 64:1536 + (i + 1) * 64, :] = o[3, :, 192:256].T
    return out

